# revision 20
# baseline (speedup 1.0000x reference)
"""AtomicDipolesMACE on 8 Trainium2 NeuronCores.

Sharding: edges partitioned by receiver node block (core k owns nodes
[k*1024,(k+1)*1024)), sorted by receiver window; scatter-add via one-hot
matmuls accumulating in PSUM per 128-node window.  hh = feats@W_up tables
are gathered per-edge via indirect DMA from DRAM (bf16 rows); layer 1 only
needs the m=0 block (hh1 is zero for m>0); layer 2 needs the full table,
assembled per-core and AllGathered between layers.
"""
import math
import numpy as np
import sys

sys.path.insert(0, "/opt/trn_rl_repo")

from contextlib import ExitStack

import concourse.bass as bass
import concourse.mybir as mybir
import concourse.tile as tile
from concourse.bass import IndirectOffsetOnAxis
from concourse.bass_utils import run_bass_kernel_spmd
from concourse.masks import make_identity
import json as _json
from concourse import bass_utils as _bu
from concourse import bass2jax as _b2j


def _legalize_waits(bir):
    """Walrus codegen allows one sync wait per instruction; hoist extras
    into single-wait NoOps placed immediately before on the same engine."""
    cnt = 0
    for fn in bir["functions"]:
        for blk in fn["blocks"]:
            out = []
            for inst in blk["instructions"]:
                si = inst.get("sync_info")
                if si and si.get("on_wait") and len(si["on_wait"]) > 1:
                    ws = si["on_wait"]
                    for w in ws[:-1]:
                        cnt += 1
                        out.append({"debug": inst.get("debug", 0),
                                    "engine": inst["engine"], "ins": [],
                                    "outs": [], "name": f"I-wsplit-{cnt}",
                                    "opcode": "NoOp",
                                    "sync_info": {"on_update": [],
                                                  "on_wait": [w]}})
                    si["on_wait"] = [ws[-1]]
                out.append(inst)
            blk["instructions"] = out
    return bir


if not getattr(_bu.compile_bir_kernel, "_wait_legalized", False):
    _orig_cbk = _bu.compile_bir_kernel

    def _patched_cbk(bir_json, tmpdir, neff_name="file.neff"):
        d = _json.loads(bir_json)
        _legalize_waits(d)
        return _orig_cbk(_json.dumps(d).encode(), tmpdir, neff_name)

    _patched_cbk._wait_legalized = True
    _bu.compile_bir_kernel = _patched_cbk
    _b2j.compile_bir_kernel = _patched_cbk

F32 = mybir.dt.float32
BF16 = mybir.dt.bfloat16
I32 = mybir.dt.int32

N, E, F, NE, G = 8192, 131072, 64, 10, 64
R_MAX = 5.0
NB = 8
M = 16
MF = M * F
NCORES = 8
NLOC = N // NCORES
NW = NLOC // 128
AVG_NEIGH = 16.0

AF = mybir.ActivationFunctionType
OP = mybir.AluOpType

_CACHE = {}


# ----------------------------------------------------------------------------
def _host_prep(positions, node_attrs, shifts, charges, W_emb, W_r1, W_r2, W_r3,
               W_r4, W_up, W_mix, W_sc, w_read, edge_index, batch):
    snd = edge_index[0].astype(np.int64)
    rcv = edge_index[1].astype(np.int64)

    win_of_edge = rcv // 128
    counts = np.bincount(win_of_edge, minlength=N // 128)
    W_E = int(math.ceil(counts.max() / 128.0) * 128)
    NT_W = W_E // 128
    NT = NW * NT_W
    EP = NT * 128

    order = np.argsort(rcv, kind="stable")
    snd_s, rcv_s = snd[order], rcv[order]
    shifts_s = shifts[order]

    bd = lambda Wm: np.block(
        [[Wm, np.zeros((F, F), np.float32)],
         [np.zeros((F, F), np.float32), Wm]]).astype(np.float32)

    def sread(wr):
        s = np.zeros((2, 128, 3), np.float32)
        s[0, 64:128, 0] = wr
        s[1, 0:64, 1] = wr
        s[1, 64:128, 2] = wr
        return np.ascontiguousarray(np.concatenate([s[0], s[1]], axis=1))

    rep = np.zeros((64, 128), np.float32)
    rep[np.arange(64), np.arange(64)] = 1.0
    rep[np.arange(64), 64 + np.arange(64)] = 1.0

    common = dict(
        positions_t=np.ascontiguousarray(positions, np.float32),
        na_T=np.ascontiguousarray(node_attrs.T.astype(np.float32)),
        W_emb=np.ascontiguousarray(W_emb, np.float32),
        W_up0=np.ascontiguousarray(W_up[0], np.float32),
        W_r1=np.ascontiguousarray(W_r1, np.float32),
        W_r2=np.ascontiguousarray(W_r2, np.float32),
        W_r3=np.ascontiguousarray(W_r3, np.float32),
        W_r4=np.ascontiguousarray(W_r4, np.float32),
        bdW_up1=bd(W_up[1]), bdW_mix0=bd(W_mix[0]), bdW_mix1=bd(W_mix[1]),
        W_sc0=np.ascontiguousarray(W_sc[0], np.float32),
        W_sc1=np.ascontiguousarray(W_sc[1], np.float32),
        sread0=sread(w_read[0]), sread1=sread(w_read[1]),
        rep_m=rep,
    )

    in_maps = []
    for c in range(NCORES):
        snd_c = np.zeros(EP, np.int32)
        rcvg_c = np.zeros(EP, np.int32)
        rrel_c = np.full(EP, -1.0, np.float32)
        shf_c = np.zeros((EP, 3), np.float32)
        sel = (rcv_s // NLOC) == c
        sndc, rcvc, shfc = snd_s[sel], rcv_s[sel], shifts_s[sel]
        wloc = (rcvc % NLOC) // 128
        for w in range(NW):
            m = wloc == w
            k = int(m.sum())
            base = w * W_E
            snd_c[base:base + k] = sndc[m]
            rcvg_c[base:base + k] = rcvc[m]
            rrel_c[base:base + k] = (rcvc[m] % NLOC - w * 128).astype(np.float32)
            shf_c[base:base + k] = shfc[m]

        def grid(a):
            return np.ascontiguousarray(a.reshape(NT, 128).T)

        lo, hi = c * NLOC, (c + 1) * NLOC
        boh = np.zeros((NLOC, G), np.float32)
        boh[np.arange(NLOC), batch[lo:hi].astype(np.int64)] = 1.0
        boh = np.ascontiguousarray(
            boh.reshape(NW, 128, G).transpose(1, 0, 2).reshape(128, NW * G))

        posg = np.ascontiguousarray(
            positions[lo:hi].reshape(NW, 128, 3).transpose(1, 0, 2)
            .reshape(128, NW * 3).astype(np.float32))
        chg = np.ascontiguousarray(
            charges[lo:hi].reshape(NW, 128).T.astype(np.float32))

        m = dict(common)
        m.update(
            snd_idx=grid(snd_c), rcvg_idx=grid(rcvg_c), rcv_rel=grid(rrel_c),
            shf=np.ascontiguousarray(
                shf_c.reshape(NT, 128, 3).transpose(1, 2, 0).reshape(128, 3 * NT)),
            na_loc_T=np.ascontiguousarray(node_attrs[lo:hi].T.astype(np.float32)),
            pos_loc=posg, chg_loc=chg, boh=boh,
        )
        in_maps.append(m)
    return in_maps, W_E, NT


# ----------------------------------------------------------------------------
def _build(NT):
    EP = NT * 128
    NT_W = NT // NW
    nc = bass.Bass(num_devices=NCORES)

    ei = lambda nm, sh, dt=F32: nc.dram_tensor(nm, sh, dt, kind="ExternalInput")
    snd_idx = ei("snd_idx", [128, NT], I32)
    rcvg_idx = ei("rcvg_idx", [128, NT], I32)
    rcv_rel = ei("rcv_rel", [128, NT])
    shf = ei("shf", [128, 3 * NT])
    positions_t = ei("positions_t", [N, 3])
    na_T = ei("na_T", [NE, N])
    na_loc_T = ei("na_loc_T", [NE, NLOC])
    pos_loc = ei("pos_loc", [128, NW * 3])
    chg_loc = ei("chg_loc", [128, NW])
    boh = ei("boh", [128, NW * G])
    W_emb = ei("W_emb", [NE, F])
    W_up0 = ei("W_up0", [F, F])
    W_r1 = ei("W_r1", [NB, F]); W_r2 = ei("W_r2", [F, F])
    W_r3 = ei("W_r3", [F, F]); W_r4 = ei("W_r4", [F, F])
    bdW_up1 = ei("bdW_up1", [128, 128])
    bdW_mix0 = ei("bdW_mix0", [128, 128]); bdW_mix1 = ei("bdW_mix1", [128, 128])
    W_sc0 = ei("W_sc0", [NE, F]); W_sc1 = ei("W_sc1", [NE, F])
    sread0 = ei("sread0", [128, 6]); sread1 = ei("sread1", [128, 6])
    rep_m = ei("rep_m", [F, 128])

    dip_out = nc.dram_tensor("dip_out", [3, NLOC], F32, kind="ExternalOutput")
    tot_out = nc.dram_tensor("tot_out", [G, 3], F32, kind="ExternalOutput")

    t1_dram = nc.dram_tensor("t1_dram", [N, F], BF16, kind="Internal")
    hh2_slice = nc.dram_tensor("hh2_slice", [NLOC, MF], BF16, kind="Internal")
    hh2_full = nc.dram_tensor("hh2_full", [N, MF], BF16, kind="Internal",
                              addr_space="Shared")

    s3, s5, s15 = math.sqrt(3.), math.sqrt(5.), math.sqrt(15.)
    s7, s105 = math.sqrt(7.), math.sqrt(105.)
    s35_8, s21_8 = math.sqrt(35. / 8.), math.sqrt(21. / 8.)

    ctx = ExitStack()
    with ctx:
        tc = ctx.enter_context(tile.TileContext(nc, num_cores=NCORES))
        sb = ctx.enter_context(tc.tile_pool(name="sb", bufs=1))
        sbw = ctx.enter_context(tc.tile_pool(name="sbw", bufs=2))

        TT = nc.vector.tensor_tensor
        ACT = nc.scalar.activation
        MM = nc.tensor.matmul

        def load(pool, src, dt=None):
            dt = dt or src.dtype
            nm = f"ld_{src.name}"
            t = pool.tile(list(src.shape), dt, name=nm, tag=nm)
            nc.gpsimd.dma_start(out=t[:], in_=src[:])
            return t

        snd_s = load(sb, snd_idx)
        rcvg_s = load(sb, rcvg_idx)
        rrel_s = load(sb, rcv_rel)
        shf_s = load(sb, shf)
        naT_s = load(sb, na_T, BF16)
        naloc_s = load(sb, na_loc_T, BF16)
        posl_s = load(sb, pos_loc)
        chg_s = load(sb, chg_loc)
        boh_s = load(sb, boh)
        Wemb_s = load(sb, W_emb, BF16)
        Wup0_s = load(sb, W_up0, BF16)
        Wr_s = [load(sb, W_r1, BF16), load(sb, W_r2, BF16),
                load(sb, W_r3, BF16), load(sb, W_r4, BF16)]
        bdup1_s = load(sb, bdW_up1, BF16)
        bdmix_s = [load(sb, bdW_mix0, BF16), load(sb, bdW_mix1, BF16)]
        Wsc_s = [load(sb, W_sc0, BF16), load(sb, W_sc1, BF16)]
        sread_s = [load(sb, sread0, BF16), load(sb, sread1, BF16)]
        rep_s = load(sb, rep_m, BF16)

        ident = sb.tile([128, 128], F32)
        make_identity(nc, ident[:])
        identb = sb.tile([128, 128], BF16)
        nc.vector.tensor_copy(identb[:], ident[:])
        iota_i = sb.tile([128, 128], I32)
        nc.gpsimd.iota(iota_i[:], pattern=[[1, 128]], channel_multiplier=0)
        iota_f = sb.tile([128, 128], F32)
        nc.vector.tensor_copy(iota_f[:], iota_i[:])

        cp_all = sb.tile([128, NW * 3], F32)
        TT(out=cp_all[:], in0=posl_s[:],
           in1=chg_s[:].rearrange("p (w o) -> p w o", o=1)
               .to_broadcast([128, NW, 3]), op=OP.mult)

        featsA = sb.tile([128, NW * 8 * 128], BF16)
        featsB = sb.tile([128, NW * 8 * 128], BF16)
        nc.vector.memset(featsA[:], 0.0)

        # ---- phase 1: t1 table + u_loc + sc multipliers ----
        t1v = t1_dram[:, :].rearrange("(t p) g -> p t g", p=128)
        t1asm = sb.tile([128, 64 * F], BF16)
        with tc.tile_pool(name="ps1", bufs=2, space="PSUM") as ps1:
            for ch in range(N // 512):
                u_ps = ps1.tile([F, 512], F32, tag="ups")
                MM(out=u_ps[:], lhsT=Wemb_s[:],
                   rhs=naT_s[:, ch * 512:(ch + 1) * 512], start=True, stop=True)
                u_sb = sbw.tile([F, 512], BF16, tag="usb")
                nc.scalar.copy(u_sb[:], u_ps[:])
                t1_ps = ps1.tile([F, 512], F32, tag="t1ps")
                MM(out=t1_ps[:], lhsT=Wup0_s[:], rhs=u_sb[:], start=True, stop=True)
                t1_sb = sbw.tile([F, 512], BF16, tag="t1sb")
                nc.scalar.copy(t1_sb[:], t1_ps[:])
                for q in range(4):
                    tp = ps1.tile([128, F], BF16, tag="t1tp")
                    nc.tensor.transpose(out=tp[:],
                                        in_=t1_sb[:, q * 128:(q + 1) * 128],
                                        identity=identb[0:64, 0:64])
                    nc.vector.tensor_copy(
                        t1asm[:, (ch * 4 + q) * F:(ch * 4 + q + 1) * F], tp[:])

            nc.sync.dma_start(
                out=t1v[:],
                in_=t1asm[:].rearrange("p (t g) -> p t g", g=F))

            for ch in range(NLOC // 512):
                ul_ps = ps1.tile([F, 512], F32, tag="ups")
                MM(out=ul_ps[:], lhsT=Wemb_s[:],
                   rhs=naloc_s[:, ch * 512:(ch + 1) * 512], start=True, stop=True)
                for w2 in range(4):
                    w = ch * 4 + w2
                    nc.scalar.copy(
                        featsA[0:64, (w * 8) * 128:(w * 8) * 128 + 128],
                        ul_ps[:, w2 * 128:(w2 + 1) * 128])

            screp = []
            for li in range(2):
                r_sb = sb.tile([128, NLOC], BF16, name=f"screp{li}", tag=f"screp{li}")
                for q in range(2):
                    s_ps = ps1.tile([F, 512], F32, tag="ups")
                    MM(out=s_ps[:], lhsT=Wsc_s[li][:],
                       rhs=naloc_s[:, q * 512:(q + 1) * 512], start=True, stop=True)
                    s_sb = sbw.tile([F, 512], BF16, tag="usb")
                    nc.scalar.copy(s_sb[:], s_ps[:])
                    r_ps = ps1.tile([128, 512], F32, tag="reps")
                    MM(out=r_ps[:], lhsT=rep_s[:], rhs=s_sb[:], start=True,
                       stop=True)
                    nc.scalar.copy(r_sb[:, q * 512:(q + 1) * 512], r_ps[:])
                screp.append(r_sb)

        # ---- phase 2: edge geometry on [128, NT] grids ----
        vec = sb.tile([128, 3 * NT], F32)          # (c, t) layout
        vv = vec[:].rearrange("p (c t) -> p c t", c=3)
        for t in range(NT):
            pr = sbw.tile([128, 3], F32, tag="pr")
            pst = sbw.tile([128, 3], F32, tag="pst")
            nc.gpsimd.indirect_dma_start(
                out=pr[:], out_offset=None, in_=positions_t[:],
                in_offset=IndirectOffsetOnAxis(ap=rcvg_s[:, t:t + 1], axis=0))
            nc.gpsimd.indirect_dma_start(
                out=pst[:], out_offset=None, in_=positions_t[:],
                in_offset=IndirectOffsetOnAxis(ap=snd_s[:, t:t + 1], axis=0))
            TT(out=vv[:, :, t], in0=pr[:], in1=pst[:], op=OP.subtract)
        TT(out=vec[:], in0=vec[:], in1=shf_s[:], op=OP.add)

        vx, vy, vz = vec[:, 0:NT], vec[:, NT:2 * NT], vec[:, 2 * NT:3 * NT]
        gr = lambda nm: sb.tile([128, NT], F32, name=nm, tag=nm)
        r2, tmp, tmp2 = gr('r2'), gr('tmp'), gr('tmp2')
        TT(out=r2[:], in0=vx, in1=vx, op=OP.mult)
        TT(out=tmp[:], in0=vy, in1=vy, op=OP.mult)
        TT(out=r2[:], in0=r2[:], in1=tmp[:], op=OP.add)
        TT(out=tmp[:], in0=vz, in1=vz, op=OP.mult)
        TT(out=r2[:], in0=r2[:], in1=tmp[:], op=OP.add)
        r = gr('r')
        ACT(out=r[:], in_=r2[:], func=AF.Sqrt)
        rp = gr('rp')
        nc.vector.tensor_scalar_add(rp[:], r[:], 1e-9)
        rinv = gr('rinv')
        nc.vector.reciprocal(rinv[:], rp[:])
        x, y, z = gr('x'), gr('y'), gr('z')
        TT(out=x[:], in0=vx, in1=rinv[:], op=OP.mult)
        TT(out=y[:], in0=vy, in1=rinv[:], op=OP.mult)
        TT(out=z[:], in0=vz, in1=rinv[:], op=OP.mult)

        sh = sb.tile([128, M * NT], BF16)          # (m, t) layout
        shv = sh[:].rearrange("p (m t) -> p m t", m=M)
        shm = lambda m: sh[:, m * NT:(m + 1) * NT]
        nc.vector.memset(shm(0), 1.0)
        nc.scalar.mul(shm(1), x[:], s3)
        nc.scalar.mul(shm(2), y[:], s3)
        nc.scalar.mul(shm(3), z[:], s3)
        xy, x2, y2, z2, d = gr('xy'), gr('x2'), gr('y2'), gr('z2'), gr('d')
        TT(out=xy[:], in0=x[:], in1=y[:], op=OP.mult)
        nc.scalar.mul(shm(4), xy[:], s15)
        TT(out=tmp[:], in0=y[:], in1=z[:], op=OP.mult)
        nc.scalar.mul(shm(5), tmp[:], s15)
        TT(out=z2[:], in0=z[:], in1=z[:], op=OP.mult)
        ACT(out=shm(6), in_=z2[:], func=AF.Copy, scale=1.5 * s5, bias=-0.5 * s5)
        TT(out=tmp[:], in0=x[:], in1=z[:], op=OP.mult)
        nc.scalar.mul(shm(7), tmp[:], s15)
        TT(out=x2[:], in0=x[:], in1=x[:], op=OP.mult)
        TT(out=y2[:], in0=y[:], in1=y[:], op=OP.mult)
        TT(out=d[:], in0=x2[:], in1=y2[:], op=OP.subtract)
        nc.scalar.mul(shm(8), d[:], 0.5 * s15)
        ACT(out=tmp[:], in_=x2[:], func=AF.Copy, scale=3.0)
        TT(out=tmp[:], in0=tmp[:], in1=y2[:], op=OP.subtract)
        TT(out=tmp[:], in0=tmp[:], in1=y[:], op=OP.mult)
        nc.scalar.mul(shm(9), tmp[:], s35_8)
        TT(out=tmp[:], in0=xy[:], in1=z[:], op=OP.mult)
        nc.scalar.mul(shm(10), tmp[:], s105)
        q5 = gr('q5')
        ACT(out=q5[:], in_=z2[:], func=AF.Copy, scale=5.0, bias=-1.0)
        TT(out=tmp[:], in0=y[:], in1=q5[:], op=OP.mult)
        nc.scalar.mul(shm(11), tmp[:], s21_8)
        ACT(out=tmp[:], in_=z2[:], func=AF.Copy, scale=5.0, bias=-3.0)
        TT(out=tmp[:], in0=tmp[:], in1=z[:], op=OP.mult)
        nc.scalar.mul(shm(12), tmp[:], 0.5 * s7)
        TT(out=tmp[:], in0=x[:], in1=q5[:], op=OP.mult)
        nc.scalar.mul(shm(13), tmp[:], s21_8)
        TT(out=tmp[:], in0=d[:], in1=z[:], op=OP.mult)
        nc.scalar.mul(shm(14), tmp[:], 0.5 * s105)
        ACT(out=tmp[:], in_=x2[:], func=AF.Copy, scale=3.0)
        TT(out=tmp[:], in0=tmp[:], in1=y2[:], op=OP.subtract)
        TT(out=tmp[:], in0=tmp[:], in1=x[:], op=OP.mult)
        nc.scalar.mul(shm(15), tmp[:], s35_8)

        # radial basis
        u, u5 = gr('u'), gr('u5')
        nc.scalar.mul(u[:], r[:], 1.0 / R_MAX)
        TT(out=tmp[:], in0=u[:], in1=u[:], op=OP.mult)
        TT(out=tmp2[:], in0=tmp[:], in1=tmp[:], op=OP.mult)
        TT(out=u5[:], in0=tmp2[:], in1=u[:], op=OP.mult)
        ACT(out=tmp[:], in_=u[:], func=AF.Copy, scale=-15.0, bias=35.0)
        TT(out=tmp[:], in0=tmp[:], in1=u[:], op=OP.mult)
        ACT(out=tmp[:], in_=tmp[:], func=AF.Copy, scale=1.0, bias=-21.0)
        TT(out=tmp[:], in0=tmp[:], in1=u5[:], op=OP.mult)
        ACT(out=tmp[:], in_=tmp[:], func=AF.Copy, scale=1.0, bias=1.0)
        mask = gr('mask')
        nc.vector.tensor_scalar(out=mask[:], in0=u[:], scalar1=1.0,
                                scalar2=None, op0=OP.is_lt)
        TT(out=tmp[:], in0=tmp[:], in1=mask[:], op=OP.mult)
        g = gr('g')
        TT(out=g[:], in0=tmp[:], in1=rinv[:], op=OP.mult)
        nc.scalar.mul(g[:], g[:], math.sqrt(2.0 / R_MAX))
        rb = sb.tile([128, NB * NT], F32)          # (n, t) layout
        rbv = rb[:].rearrange("p (n t) -> p n t", n=NB)
        ti = sb.tile([128, NT], I32)
        half = gr('half')
        for n in range(1, NB + 1):
            nc.scalar.mul(tmp[:], r[:], n / (2.0 * R_MAX))
            nc.vector.tensor_copy(ti[:], tmp[:])
            nc.vector.tensor_copy(tmp2[:], ti[:])
            TT(out=tmp[:], in0=tmp[:], in1=tmp2[:], op=OP.subtract)
            nc.vector.tensor_scalar(out=half[:], in0=tmp[:], scalar1=0.5,
                                    scalar2=None, op0=OP.is_gt)
            TT(out=tmp[:], in0=tmp[:], in1=half[:], op=OP.subtract)
            ACT(out=tmp[:], in_=tmp[:], func=AF.Sin, scale=2.0 * math.pi)
            TT(out=rbv[:, n - 1, :], in0=tmp[:], in1=g[:], op=OP.mult)

        # ---- phase 3: radial MLP -> edge-major weights w_em [128,(t,g)] ----
        w_em = sb.tile([128, NT * F], BF16)
        with tc.tile_pool(name="ps3", bufs=2, space="PSUM") as ps3:
            for ch in range(EP // 512):
                rbT = sbw.tile([NB, 512], BF16, tag="rbT")
                for q in range(4):
                    t = ch * 4 + q
                    tp = ps3.tile([NB, 128], F32, tag="rbtp")
                    nc.tensor.transpose(out=tp[:], in_=rbv[:, :, t],
                                        identity=ident[:])
                    nc.scalar.copy(rbT[:, q * 128:(q + 1) * 128], tp[:])
                h = rbT
                for li in range(4):
                    h_ps = ps3.tile([F, 512], F32, tag="hps")
                    MM(out=h_ps[:], lhsT=Wr_s[li][:], rhs=h[:], start=True,
                       stop=True)
                    h2 = sbw.tile([F, 512], BF16, tag=f"h{li}")
                    if li < 3:
                        ACT(out=h2[:], in_=h_ps[:], func=AF.Silu)
                    else:
                        nc.scalar.copy(h2[:], h_ps[:])
                    h = h2
                for q in range(4):
                    t = ch * 4 + q
                    tp2 = ps3.tile([128, F], BF16, tag="wtp")
                    nc.tensor.transpose(out=tp2[:],
                                        in_=h[:, q * 128:(q + 1) * 128],
                                        identity=identb[0:64, 0:64])
                    nc.vector.tensor_copy(w_em[:, t * F:(t + 1) * F], tp2[:])

        dipA = sb.tile([3, NLOC], F32)
        dipB = sb.tile([3, NLOC], F32)
        hh2asm = sb.tile([128, NW * MF], BF16)

        # ---- layers ----
        psA = ctx.enter_context(tc.tile_pool(name="psA", bufs=2, space="PSUM"))
        psN = ctx.enter_context(tc.tile_pool(name="psN", bufs=2, space="PSUM"))
        psD = ctx.enter_context(tc.tile_pool(name="psD", bufs=1, space="PSUM"))

        for li in range(2):
            f_in = featsA if li == 0 else featsB
            f_out = featsB if li == 0 else featsA
            for w in range(NW):
                agg = psA.tile([128, MF], F32, tag="agg")
                for t2 in range(NT_W):
                    t = w * NT_W + t2
                    msg = sbw.tile([128, MF], BF16, tag="msg")
                    msgv = msg[:].rearrange("p (m f) -> p m f", m=M)
                    oh = sbw.tile([128, 128], BF16, tag="oh")
                    TT(out=oh[:],
                       in0=rrel_s[:, t:t + 1].to_broadcast([128, 128]),
                       in1=iota_f[:], op=OP.is_equal)
                    if li == 0:
                        h0 = sbw.tile([128, F], BF16, tag="h0")
                        nc.gpsimd.indirect_dma_start(
                            out=h0[:], out_offset=None, in_=t1_dram[:],
                            in_offset=IndirectOffsetOnAxis(
                                ap=snd_s[:, t:t + 1], axis=0))
                        a1 = sbw.tile([128, F], BF16, tag="a1")
                        TT(out=a1[:], in0=h0[:], in1=w_em[:, t * F:(t + 1) * F],
                           op=OP.mult)
                        TT(out=msgv,
                           in0=a1[:].rearrange("p (o f) -> p o f", o=1)
                               .to_broadcast([128, M, F]),
                           in1=shv[:, :, t].to_broadcast([128, M, F]),
                           op=OP.mult)
                        TT(out=msg[:, 0:F], in0=msg[:, 0:F], in1=a1[:], op=OP.add)
                    else:
                        hsd = sbw.tile([128, MF], BF16, tag="hsd")
                        nc.gpsimd.indirect_dma_start(
                            out=hsd[:], out_offset=None, in_=hh2_full[:],
                            in_offset=IndirectOffsetOnAxis(
                                ap=snd_s[:, t:t + 1], axis=0))
                        a1 = sbw.tile([128, F], BF16, tag="a1")
                        TT(out=a1[:], in0=hsd[:, 0:F],
                           in1=w_em[:, t * F:(t + 1) * F], op=OP.mult)
                        TT(out=msgv,
                           in0=hsd[:].rearrange("p (m f) -> p m f", m=M),
                           in1=w_em[:, t * F:(t + 1) * F]
                               .rearrange("p (o f) -> p o f", o=1)
                               .to_broadcast([128, M, F]),
                           op=OP.mult)
                        ash = sbw.tile([128, MF], BF16, tag="ash")
                        TT(out=ash[:].rearrange("p (m f) -> p m f", m=M),
                           in0=a1[:].rearrange("p (o f) -> p o f", o=1)
                               .to_broadcast([128, M, F]),
                           in1=shv[:, :, t].to_broadcast([128, M, F]),
                           op=OP.mult)
                        TT(out=msg[:], in0=msg[:], in1=ash[:], op=OP.add)
                    for hf in range(2):
                        MM(out=agg[:, hf * 512:(hf + 1) * 512],
                           lhsT=oh[:],
                           rhs=msg[:, hf * 512:(hf + 1) * 512],
                           start=(t2 == 0), stop=(t2 == NT_W - 1),
                           skip_group_check=True)

                # node update for window w
                agg_sb = sbw.tile([128, MF], BF16, tag="aggsb")
                nc.scalar.mul(agg_sb[:], agg[:], 1.0 / AVG_NEIGH)
                fnew = []
                for c in range(8):
                    tp = psN.tile([128, 128], BF16, tag="nps")
                    nc.tensor.transpose(out=tp[:],
                                        in_=agg_sb[:, c * 128:(c + 1) * 128],
                                        identity=identb[:])
                    aggT = sbw.tile([128, 128], BF16, tag="aggTs")
                    nc.vector.tensor_copy(aggT[:], tp[:])
                    mx = psN.tile([128, 128], F32, tag="nps")
                    MM(out=mx[:], lhsT=bdmix_s[li][:], rhs=aggT[:],
                       start=True, stop=True, skip_group_check=True)
                    sc = sbw.tile([128, 128], BF16, tag="sc")
                    TT(out=sc[:],
                       in0=f_in[:, (w * 8 + c) * 128:(w * 8 + c) * 128 + 128],
                       in1=screp[li][:, w * 128:(w + 1) * 128], op=OP.mult)
                    fn = sbw.tile([128, 128], BF16, tag=f"fn{c}")
                    TT(out=fn[:], in0=mx[:], in1=sc[:], op=OP.add)
                    fnew.append(fn)
                f0r_ps = psN.tile([128, 128], F32, tag="nps")
                MM(out=f0r_ps[:], lhsT=rep_s[:], rhs=fnew[0][0:64, :],
                   start=True, stop=True, skip_group_check=True)
                f0h = sbw.tile([128, 128], BF16, tag="f0h")
                nc.scalar.mul(f0h[:], f0r_ps[:], 0.5)
                dipp = psD.tile([3, 128], F32, tag="dipp")
                for c in range(8):
                    prod = sbw.tile([128, 128], BF16, tag="prod")
                    TT(out=prod[:], in0=fnew[c][:], in1=f0h[:], op=OP.mult)
                    fo = f_out[:, (w * 8 + c) * 128:(w * 8 + c) * 128 + 128]
                    TT(out=fo, in0=fnew[c][:], in1=prod[:], op=OP.add)
                    if c < 2:
                        MM(out=dipp[:], lhsT=sread_s[li][:, c * 3:(c + 1) * 3],
                           rhs=fo, start=(c == 0), stop=(c == 1),
                           skip_group_check=True)
                    if li == 0:
                        hhp = psN.tile([128, 128], F32, tag="nps")
                        MM(out=hhp[:], lhsT=bdup1_s[:], rhs=fo,
                           start=True, stop=True, skip_group_check=True)
                        hhs = sbw.tile([128, 128], BF16, tag="hhs")
                        nc.scalar.copy(hhs[:], hhp[:])
                        hht = psN.tile([128, 128], BF16, tag="nps")
                        nc.tensor.transpose(out=hht[:], in_=hhs[:],
                                            identity=identb[:])
                        nc.vector.tensor_copy(
                            hh2asm[:, w * MF + c * 128:w * MF + (c + 1) * 128],
                            hht[:])
                if li == 0:
                    nc.scalar.copy(dipA[:, w * 128:(w + 1) * 128], dipp[:])
                else:
                    nc.scalar.copy(dipB[:, w * 128:(w + 1) * 128], dipp[:])
            if li == 0:
                nc.sync.dma_start(
                    out=hh2_slice[:, :].rearrange("(w p) c -> p w c", p=128),
                    in_=hh2asm[:].rearrange("p (w c) -> p w c", c=MF))
                nc.gpsimd.collective_compute(
                    kind="AllGather", op=OP.bypass,
                    replica_groups=[list(range(NCORES))],
                    ins=[hh2_slice[:]], outs=[hh2_full[:]])

        # ---- outputs ----
        dip = sb.tile([3, NLOC], F32)
        TT(out=dip[:], in0=dipA[:], in1=dipB[:], op=OP.add)
        nc.sync.dma_start(out=dip_out[:], in_=dip[:])
        totp = psD.tile([G, 3], F32, tag="totp")
        for w in range(NW):
            dT = psN.tile([128, 3], F32, tag="nps")
            nc.tensor.transpose(out=dT[:], in_=dip[:, w * 128:(w + 1) * 128],
                                identity=ident[0:3, 0:3])
            dT_sb = sbw.tile([128, 3], F32, tag="dTs")
            nc.vector.tensor_copy(dT_sb[:], dT[:])
            val = sbw.tile([128, 3], F32, tag="val")
            TT(out=val[:], in0=dT_sb[:], in1=cp_all[:, w * 3:(w + 1) * 3],
               op=OP.add)
            MM(out=totp[:], lhsT=boh_s[:, w * G:(w + 1) * G], rhs=val[:],
               start=(w == 0), stop=(w == NW - 1), skip_group_check=True)
        tot_sb = sb.tile([G, 3], F32)
        nc.scalar.copy(tot_sb[:], totp[:])
        nc.sync.dma_start(out=tot_out[:], in_=tot_sb[:])

    return nc


# ----------------------------------------------------------------------------
def kernel(**inputs):
    inputs = {k: np.asarray(v) for k, v in inputs.items()}
    in_maps, W_E, NT = _host_prep(**inputs)

    key = ("nc", NT)
    if key not in _CACHE:
        _CACHE[key] = _build(NT)
    nc = _CACHE[key]

    res = run_bass_kernel_spmd(nc, in_maps, core_ids=list(range(NCORES)))
    dip = np.zeros((N, 3), np.float32)
    tot = np.zeros((G, 3), np.float32)
    for c in range(NCORES):
        out = res.results[c]
        dip[c * NLOC:(c + 1) * NLOC] = np.asarray(out["dip_out"]).T
        tot += np.asarray(out["tot_out"])
    return tot, dip
